# revision 9
# baseline (speedup 1.0000x reference)
"""Chunked local attention (B=4, S=8192, D=1024, H=16, Dh=64, C=256, W=64)
on 8 Trainium2 NeuronCores.

Sharding: data-parallel over the 128 (batch x chunk) units -> 16 chunks/core.
The host passes each core its x shard pre-transposed in bf16 ([D, 16*256]);
weights are replicated bf16.

All matmuls run single-pass bf16 with fp32 PSUM accumulation (harness
tolerance 2e-2, this lands ~4e-3). Chunks are processed in PAIRS so the
weight-stationary q/k projections stream N=512 moving columns per LDWEIGHTS
instead of N=256 — halving the q/k weight-load count, which is pure PE
overhead.

Per-core dataflow (per 512-token chunk pair):
  qT/kT = Wq^T/Wk^T @ xT      (lhsT=W, rhs=xT[512])        [dout, tok] bf16
  then per 256-token sub-chunk:
    v     = xT^T @ Wv         (lhsT=xT, rhs=Wv)            [tok, dout] bf16
            (ones column appended per head -> softmax denominator for free)
    s_h   = kT_h^T x qT_h     (head-pair 64-row groups run concurrently
                               in the PE, separate PSUM banks)
    p     = exp(0.125*s) * bandmask
    oU|den= p_h^T @ [v_h|1]   (lhsT=p_h, rhs=v_aug)  fp32  [i, 65] per head
    oN    = oU * (1/den)  per-head bcast                   [i, d] bf16
    oT    = PE-transpose(oN)  (bf16, 1 cyc/row)            [d, i] bf16
    y     = oT^T @ Wo         (lhsT=oT, rhs=Wo)            [tok, d2] bf16 out

Engine balance: PE all matmuls; ACT exps + transpose copies; DVE the
PSUM->SBUF downcasts, reciprocal, normalize, half the mask-muls; Pool the
other half of the mask-muls (Pool cannot touch PSUM on TRN2). Transpose
PSUM tiles live in the o_ps pool so the score pipeline keeps its two
banks exclusively. Band structure (j <= i <= j+64) restricts
score/prob work to the only column ranges the PV matmuls ever read.
"""

from contextlib import ExitStack

import ml_dtypes
import numpy as np

import concourse.bass as bass
import concourse.mybir as mybir
import concourse.tile as tile
from concourse import bacc
from concourse.bass_utils import run_bass_kernel_spmd
from concourse.masks import make_identity

B, S, D = 4, 8192, 1024
H, DH, C, W = 16, 64, 256, 64
NCORES = 8
NCHUNKS_TOTAL = B * (S // C)      # 128
CPC = NCHUNKS_TOTAL // NCORES     # 16 chunks per core
TPC = CPC * C                     # 4096 tokens per core
C2 = 2 * C                        # chunk-pair width
F32 = mybir.dt.float32
BF16 = mybir.dt.bfloat16
KT = D // 128                     # 8 k-tiles over the contraction dim
BF = ml_dtypes.bfloat16
WNAMES = ("wq", "wk", "wv", "wo")


def _band_mask_np():
    # maskT[slot, jj, ii] = band(j, i), layout [j, i], 192 i-cols per slot:
    # slot0: j = jj,     i = ii        (j-tile0, i in [0,192))
    # slot1: j = 128+jj, i = 64+ii    (j-tile1, i in [64,256))
    jj = np.arange(128)[:, None]
    ii = np.arange(192)[None, :]
    def band(j, i):
        return ((j <= i) & (i <= j + W)).astype(BF)
    m = np.stack([band(jj, ii), band(128 + jj, 64 + ii)])
    return np.ascontiguousarray(m)


def _emit(ctx, tc, io, n_pairs):
    nc = tc.nc
    x_d, w_d, mask_d, y_d = io

    singles = ctx.enter_context(tc.tile_pool(name="singles", bufs=1))
    xpool = ctx.enter_context(tc.tile_pool(name="xpool", bufs=2))
    qkpool = ctx.enter_context(tc.tile_pool(name="qkpool", bufs=2))
    vpool = ctx.enter_context(tc.tile_pool(name="vpool", bufs=2))
    ppool = ctx.enter_context(tc.tile_pool(name="ppool", bufs=12))
    opool = ctx.enter_context(tc.tile_pool(name="opool", bufs=2))
    otpool = ctx.enter_context(tc.tile_pool(name="otpool", bufs=2))
    ypool = ctx.enter_context(tc.tile_pool(name="ypool", bufs=3))
    dnpool = ctx.enter_context(tc.tile_pool(name="dnpool", bufs=4))

    # 8 PSUM banks: 2 qk (full-bank [128,512] accum), 2 scores (exclusive:
    # the score->exp->PV ring is the hottest chain), 2 v/y, 2 o (+transposes
    # at the chunk tail). NOTE: two matmuls with different PE row groups must
    # NOT share a PSUM bank (hardware fault, bisected 2026-08-08).
    psqk = ctx.enter_context(tc.tile_pool(name="psqk", bufs=2, space="PSUM"))
    pssc = ctx.enter_context(tc.tile_pool(name="pssc", bufs=2, space="PSUM"))
    ps512 = ctx.enter_context(tc.tile_pool(name="ps512", bufs=2, space="PSUM"))
    psbig = ctx.enter_context(tc.tile_pool(name="psbig", bufs=2, space="PSUM"))

    # --- constants / weights resident in SBUF (bf16) ---
    # Weights/mask load on the Pool (SWDGE) queues so the per-chunk x/y
    # traffic on the HWDGE queues is not stuck behind 8MB of weights at
    # kernel start.
    w_sb = {}
    for wn in WNAMES:
        kts = []
        for kt in range(KT):
            t = singles.tile([128, D], BF16, tag=f"{wn}{kt}", name=f"{wn}{kt}")
            nc.gpsimd.dma_start(
                out=t, in_=w_d[wn].ap()[kt * 128:(kt + 1) * 128, :])
            kts.append(t)
        w_sb[wn] = kts
    mask_sb = singles.tile([128, 2, 192], BF16, tag="mask")
    nc.gpsimd.dma_start(out=mask_sb, in_=mask_d.ap().rearrange("jt p i -> p jt i"))
    ident = singles.tile([128, 128], BF16, tag="ident")
    make_identity(nc, ident)

    def attention(cc, tok0, x, qT, kT):
        """One 256-token sub-chunk: v, scores, PV, normalize, transpose, y."""
        cb = cc * C     # column base of this sub-chunk inside the pair tiles
        # --- v projection: v_sb[j-par, jt, head, 65] bf16 + ones column ---
        v_sb = vpool.tile([128, 2, H, DH + 1], BF16, tag="v")
        nc.gpsimd.memset(v_sb[:, :, :, DH:], 1.0)
        for jt in range(2):
            jsl = slice(cb + jt * 128, cb + (jt + 1) * 128)
            for nn in range(2):
                ps = ps512.tile([128, 512], F32, tag="ps512")
                nsl = slice(nn * 512, (nn + 1) * 512)
                for kt in range(KT):
                    nc.tensor.matmul(ps, x[:, kt, jsl], w_sb["wv"][kt][:, nsl],
                                     start=(kt == 0), stop=(kt == KT - 1))
                nc.vector.tensor_copy(
                    out=v_sb[:, jt, nn * 8:(nn + 1) * 8, :DH],
                    in_=ps.rearrange("p (h d) -> p h d", h=8))

        # --- attention, 4 heads (one PSUM bank of [128,4,65]) at a time ---
        oN = [opool.tile([128, D], BF16, tag="oN", name=f"oN{tok0}_{i}")
              for i in range(2)]
        for qt in range(4):
            o_ps = [psbig.tile([128, 4, 2 * DH], F32, tag="obig",
                               name=f"o_ps{tok0}_{qt}_{i}") for i in range(2)]
            # head pairs (2*hm, 2*hm+1) sit on partitions 0-63 / 64-127; their
            # K=64 score matmuls use disjoint PE row groups and are emitted
            # interleaved so they run concurrently in the array.
            for pr in range(2):
                hm = qt * 2 + pr
                h0, h1 = 2 * hm, 2 * hm + 1
                lo64, hi64 = slice(0, 64), slice(64, 128)
                # Band structure (j <= i <= j+64). Each head gets ONE full
                # PSUM bank [128, 512]: j-tile0 scores (i in [0,192)) at
                # cols 0:192, j-tile1 scores (i in [64,256)) at cols
                # 256:448 — both written by the head's own row group, so
                # bank sharing is safe. One 3D exp + one mask-mul per head
                # covers both slots; every exp'd column is real data (the
                # band mask is zero on the widened i ranges).
                jsl0 = slice(cb, cb + 128)
                jsl1 = slice(cb + 128, cb + C)
                probs = {}   # head_in_pair -> [128, 2, 192] bf16 tile
                for hp, rg in ((0, lo64), (1, hi64)):
                    hh = hm
                    s_h = pssc.tile([128, 512], F32, tag="pssc",
                                    name=f"s_{tok0}_{hm}_{hp}")
                    nc.tensor.matmul(s_h[:, 0:192], kT[rg, hh, jsl0],
                                     qT[rg, hh, cb:cb + 192],
                                     start=True, stop=True)
                    nc.tensor.matmul(s_h[:, 256:448], kT[rg, hh, jsl1],
                                     qT[rg, hh, cb + 64:cb + C],
                                     start=True, stop=True)
                    s_ap = bass.AP(
                        tensor=s_h.tensor, offset=s_h.offset,
                        ap=[s_h.ap[0], [256, 2], [1, 192]])
                    p_sb = ppool.tile([128, 2, 192], BF16, tag="probs",
                                      name=f"p_{tok0}_{hm}_{hp}")
                    nc.scalar.activation(
                        out=p_sb, in_=s_ap,
                        func=mybir.ActivationFunctionType.Exp, scale=0.125)
                    meng = nc.gpsimd if hp == 0 else nc.vector
                    meng.tensor_mul(p_sb, p_sb, mask_sb)
                    probs[hp] = p_sb
                # PV (+den via ones column). i-tile0: j-tile0 only.
                # i-tile1: full-M j-tile1 matmul first (start=True covers all
                # 128 partitions), then the 64-wide j-tile0 partial (i in
                # [128,192)) accumulates into partitions 0:64.
                for hp, h in ((0, h0), (1, h1)):
                    hq = h - qt * 4
                    nc.tensor.matmul(
                        o_ps[0][:, hq, :DH + 1],
                        probs[hp][:, 0, 0:128],
                        v_sb[:, 0, h, :],
                        start=True, stop=True)
                    nc.tensor.matmul(
                        o_ps[1][64:128, hq, :DH + 1],
                        probs[hp][:, 1, 128:192],
                        v_sb[:, 1, h, :],
                        start=True, stop=True)
                    nc.tensor.matmul(
                        o_ps[1][0:64, hq, :DH + 1],
                        probs[hp][:, 1, 64:128],
                        v_sb[:, 1, h, :],
                        start=True, stop=False)
                    nc.tensor.matmul(
                        o_ps[1][0:64, hq, :DH + 1],
                        probs[hp][:, 0, 128:192],
                        v_sb[:, 0, h, :],
                        start=False, stop=True)
            # normalize this quarter: oN = oU * (1/den), fused in PSUM copy
            for it in range(2):
                denr = dnpool.tile([128, 4], F32, tag="denr")
                nc.vector.reciprocal(out=denr, in_=o_ps[it][:, :, DH])
                denr_bc = bass.AP(
                    tensor=denr.tensor, offset=denr.offset,
                    ap=[denr.ap[0], denr.ap[1], [0, DH]])
                nc.vector.tensor_mul(
                    oN[it][:, qt * 256:(qt + 1) * 256]
                    .rearrange("p (h d) -> p h d", h=4),
                    o_ps[it][:, :, :DH],
                    denr_bc)

        # --- transpose oN -> oT [dout-par, dt, i], bf16 1 cyc/row ---
        oT = otpool.tile([128, KT, C], BF16, tag="oT")
        for dt in range(KT):
            ps = psbig.tile([128, C], BF16, tag="obig", name=f"tp_{tok0}_{dt}")
            for it in range(2):
                nc.tensor.transpose(ps[:, it * 128:(it + 1) * 128],
                                    oN[it][:, dt * 128:(dt + 1) * 128], ident)
            nc.scalar.copy(out=oT[:, dt, :], in_=ps)

        # --- y projection + store (bf16 out, host upcasts) ---
        for it in range(2):
            isl = slice(it * 128, (it + 1) * 128)
            for nn in range(2):
                ps = ps512.tile([128, 512], F32, tag="ps512")
                nsl = slice(nn * 512, (nn + 1) * 512)
                for dt in range(KT):
                    nc.tensor.matmul(ps, oT[:, dt, isl], w_sb["wo"][dt][:, nsl],
                                     start=(dt == 0), stop=(dt == KT - 1))
                y_sb = ypool.tile([128, 512], BF16, tag="y")
                nc.vector.tensor_copy(out=y_sb, in_=ps)
                nc.sync.dma_start(
                    out=y_d.ap()[tok0 + it * 128:tok0 + (it + 1) * 128, nsl],
                    in_=y_sb)

    for cp in range(n_pairs):
        tok0 = cp * C2
        # --- load xT chunk pair: [128, KT, 512] bf16 ---
        x = xpool.tile([128, KT, C2], BF16, tag="x")
        nc.sync.dma_start(
            out=x,
            in_=x_d.ap()[:, tok0:tok0 + C2].rearrange("(kt p) t -> p kt t", p=128))

        # --- qT, kT projections for both chunks: N=512 per LDWEIGHTS ---
        qk = {}
        for wn in ("wq", "wk"):
            t = qkpool.tile([128, KT, C2], BF16, tag=wn + "T", name=wn + "T")
            for m in range(KT):
                ps = psqk.tile([128, C2], F32, tag="psqk")
                for kt in range(KT):
                    nc.tensor.matmul(ps, w_sb[wn][kt][:, m * 128:(m + 1) * 128],
                                     x[:, kt, :],
                                     start=(kt == 0), stop=(kt == KT - 1))
                nc.vector.tensor_copy(out=t[:, m, :], in_=ps)
            qk[wn] = t

        for cc in range(2):
            attention(cc, tok0 + cc * C, x, qk["wq"], qk["wk"])


def build(n_pairs=CPC // 2, n_cores=NCORES):
    nc = bacc.Bacc("TRN2", target_bir_lowering=False, debug=False,
                   num_devices=n_cores)
    x_d = nc.dram_tensor("xt", [D, TPC], BF16, kind="ExternalInput")
    w_d = {}
    for wn in WNAMES:
        w_d[wn] = nc.dram_tensor(wn, [D, D], BF16, kind="ExternalInput")
    mask_d = nc.dram_tensor("maskt", [2, 128, 192], BF16, kind="ExternalInput")
    y_d = nc.dram_tensor("y", [TPC, D], BF16, kind="ExternalOutput")
    io = (x_d, w_d, mask_d, y_d)
    with tile.TileContext(nc) as tc, ExitStack() as ctx:
        _emit(ctx, tc, io, n_pairs)
    nc.compile()
    return nc


def make_in_maps(x, Wq, Wk, Wv, Wo):
    xc = np.ascontiguousarray(x, np.float32).reshape(NCHUNKS_TOTAL, C, D)
    mask = _band_mask_np()
    wmap = {}
    for wn, w in zip(WNAMES, (Wq, Wk, Wv, Wo)):
        wmap[wn] = np.ascontiguousarray(np.asarray(w, np.float32).astype(BF))
    in_maps = []
    for s in range(NCORES):
        shard = xc[s * CPC:(s + 1) * CPC].reshape(TPC, D)
        xT = np.ascontiguousarray(shard.T.astype(BF))
        in_maps.append({"xt": xT, "maskt": mask, **wmap})
    return in_maps


_NC_CACHE = {}


def kernel(x, Wq, Wk, Wv, Wo):
    if "nc" not in _NC_CACHE:
        _NC_CACHE["nc"] = build()
    nc = _NC_CACHE["nc"]
    in_maps = make_in_maps(x, Wq, Wk, Wv, Wo)
    res = run_bass_kernel_spmd(nc, in_maps, core_ids=list(range(NCORES)))
    out = np.concatenate([res.results[s]["y"].astype(np.float32)
                          for s in range(NCORES)], axis=0)
    return out.reshape(B, S, D)
